# revision 33
# baseline (speedup 1.0000x reference)
"""Masked attention kernel for Trainium2, row-parallel over 8 NeuronCores.

Reference computation (per problem):
    S   = (Q @ K^T) / sqrt(D)          [NQ, NK]
    S   = where(m, S, -1e30)
    P   = softmax(S, axis=-1)
    out = P @ V                        [NQ, D]

Strategy:
  * Shard queries across 8 cores (1024 rows each); K/V/mask-columns replicated
    or sliced appropriately. No collectives.
  * Scores are computed TRANSPOSED on-chip: S_T[k, q] = sum_d K[k,d] * Qs[q,d]
    with Qs = Q/sqrt(D) pre-scaled on host. This makes the second matmul
    (P^T as lhsT, V as rhs) transpose-free.
  * Softmax without max-subtraction (scores are ~N(0,1); exp is safe in f32),
    multiplying by the 0/1 mask after exp.
  * Denominator for free: V is extended with a ones column, so PSUM column 256
    accumulates sum_k P[q,k]; normalize with reciprocal + per-partition scale.
  * bf16 matmul inputs (f32 PSUM accumulation), f32 output.
  * Mask is stored q-block-major so the qb0 pass streams only its own half;
    input DMAs are issued smallest-first in consumption order so the k-loop
    starts as soon as the first k-tile lands (~2 us after the DMA engines
    open) instead of waiting for whole-tensor transfers.
"""

import os
import sys

import numpy as np

sys.path.insert(0, "/opt/trn_rl_repo")

import ml_dtypes

NQ, NK, D = 8192, 8192, 256
NCORES = 8
QSH = NQ // NCORES          # 1024 queries per core
P = 128
KT_TILES = NK // P          # 64 key tiles
QB = 512                    # q-block (matmul moving free dim)
NQB = QSH // QB             # 2 q-blocks per core
VE = D + 1                  # V extended with ones column

_STATE = {}
LAST_RESULTS = None
TRACE = bool(os.environ.get("BASS_TRACE"))


def _build():
    import concourse.tile as tile
    from concourse import bacc, mybir

    bf16 = mybir.dt.bfloat16
    f32 = mybir.dt.float32
    u8 = mybir.dt.uint8

    nc = bacc.Bacc("TRN2", debug=False, enable_asserts=False, num_devices=NCORES)

    # All big inputs are partition-major: [128, ..., free] so chunked
    # DMAs move large contiguous per-partition ranges.
    kt_d = nc.dram_tensor("KT", [P, KT_TILES, 2 * P], bf16, kind="ExternalInput").ap()
    vt_d = nc.dram_tensor("VT", [P, KT_TILES, VE], bf16, kind="ExternalInput").ap()
    qt_d = nc.dram_tensor("QT", [P, NQB, 2, QB], bf16, kind="ExternalInput").ap()
    mt_d = nc.dram_tensor("MT", [P, NQB, KT_TILES, QB], u8, kind="ExternalInput").ap()
    out_d = nc.dram_tensor("out", [QSH, D], bf16, kind="ExternalOutput").ap()

    Exp = mybir.ActivationFunctionType.Exp
    mult = mybir.AluOpType.mult

    with tile.TileContext(nc) as tc:
        with (
            tc.tile_pool(name="singles", bufs=1) as singles,
            tc.tile_pool(name="pp", bufs=8) as pp,
            tc.tile_pool(name="outp", bufs=6) as outp,
            tc.tile_pool(name="smallp", bufs=4) as smallp,
            tc.tile_pool(name="spsum", bufs=4, space="PSUM") as spsum,
            tc.tile_pool(name="opsum", bufs=1, space="PSUM") as opsum,
        ):
            # First priming chunk on the scalar HWDGE ring, issued before
            # the ACT table load so it runs in parallel with sync's qt0.
            kt_sb = singles.tile([P, KT_TILES, 2 * P], bf16)
            nc.scalar.dma_start(out=kt_sb[:, 0:2, :], in_=kt_d[:, 0:2, :])

            # Prewarm the ACT exp table so its ~2.7us load overlaps input DMA.
            warm = singles.tile([P, 1], f32)
            nc.vector.memset(warm, 0.0)
            warm2 = smallp.tile([P, 1], f32, tag="warm2")
            nc.scalar.activation(warm2, warm, Exp)

            # Warm the PE HAM clock gate while the first input chunks stream
            # in: ~3.6us of continuous dummy matmuls so the HAM SHORT window
            # flips to 2.4 GHz right as the first k-tile's data lands. The
            # dummy matmuls read a raw (untracked, uninitialized) SBUF tensor
            # so they have no input dependencies; they target the opsum 'o0'
            # tile (whose first real use, an accumulation start, happens ~2 us
            # later) so the score-psum pool is never blocked.
            wdummy = nc.alloc_sbuf_tensor("wdummy", [P, QB], bf16).ap()
            warm_ps = opsum.tile([P, VE], f32, tag="o0", name="warm_ps")
            NWARM = 16
            for i in range(NWARM):
                nc.tensor.matmul(
                    warm_ps, lhsT=wdummy[:, 0:P], rhs=wdummy[:, 0:VE],
                    start=(i == 0), stop=(i == NWARM - 1),
                )

            qt_sb = singles.tile([P, NQB, 2, QB], bf16)
            vt_sb = singles.tile([P, KT_TILES, VE], bf16)
            mt_sb = singles.tile([P, NQB, KT_TILES, QB], u8)

            # Input DMAs on two queues, each in strict consumption order.
            # While the DMA path ramps (first ~6us it only sustains
            # ~150-250 GB/s), the sync queue carries just qt+kt — the stream
            # that gates mm1 directly (~75 GB/s steady). The mask and V
            # streams feed the lagging exp/mm2 stages; they ride gpsimd's
            # queue, gated open only after qt0 lands so packet-grain
            # round-robin can't slow the priming. Steady-state each queue
            # gets ~179 GB/s — both streams fit. qt1/mt1 are only needed by
            # the qb1 pass ~55us in, so they sit at the back of their queues.
            def kt_dma(a, b):
                nc.sync.dma_start(out=kt_sb[:, a:b, :], in_=kt_d[:, a:b, :])

            def vt_dma(a, b):
                nc.gpsimd.dma_start(
                    out=vt_sb[:, a:b, :], in_=vt_d[:, a:b, :]
                )

            def mt_dma(qh, a, b):
                nc.gpsimd.dma_start(
                    out=mt_sb[:, qh, a:b, :], in_=mt_d[:, qh, a:b, :]
                )

            nc.sync.dma_start(out=qt_sb[:, 0, :, :], in_=qt_d[:, 0, :, :])
            kt_dma(2, 6)
            kt_dma(6, 14)
            kt_dma(14, 22)
            # qb1 half of Q — not needed until the second pass.
            nc.sync.dma_start(out=qt_sb[:, 1, :, :], in_=qt_d[:, 1, :, :])
            for a, b in [(22, 30), (30, 38), (38, 46), (46, 54), (54, 64)]:
                kt_dma(a, b)

            # One real gate: a tiny gpsimd copy that waits for qt0 (reads
            # qt_sb) and scribbles into the first mask DMA's corner (the DMA
            # overwrites it). Runtime: the gpsimd engine stalls here, so its
            # whole DMA FIFO opens only after the priming lands. The
            # tile_wait_until hints keep the scheduler from hoisting later
            # gpsimd DMAs ahead of the gate in the emitted program.
            nc.gpsimd.tensor_copy(mt_sb[0:1, 0, 0, 0:2], qt_sb[0:1, 0, 0, 0:2])
            wi = 0

            def gp_seq():
                nonlocal_ms = 0.01 + 0.0005 * gp_seq.i
                gp_seq.i += 1
                return tc.tile_wait_until(nonlocal_ms)
            gp_seq.i = 0

            with gp_seq():
                mt_dma(0, 0, 2)
            with gp_seq():
                vt_dma(0, 2)
            with gp_seq():
                mt_dma(0, 2, 6)
            with gp_seq():
                vt_dma(2, 6)
            groups = [(6, 14), (14, 22), (22, 30), (30, 38),
                      (38, 46), (46, 54), (54, 64)]
            for gi, (a, b) in enumerate(groups):
                with gp_seq():
                    mt_dma(0, a, b)
                with gp_seq():
                    vt_dma(a, b)
                if gi == 4:
                    with gp_seq():
                        mt_dma(1, 0, 16)
                elif gi == 5:
                    with gp_seq():
                        mt_dma(1, 16, 32)
            with gp_seq():
                mt_dma(1, 32, 48)
            with gp_seq():
                mt_dma(1, 48, 64)

            # Software pipelining across BOTH q-block passes: mm1/exp/mask
            # run SKEW stages ahead of mm2 in one global stream, so the PE
            # never waits on the ~1.9us mm1 -> exp -> mask chain — not even
            # at the qb0/qb1 boundary (qb0's mm2 tail hides under qb1's
            # first mm1 tiles, and qb0's epilogue runs on ACT/DVE while the
            # PE streams on).
            SKEW = 4
            NSTAGE = NQB * KT_TILES
            o_ps_all = [
                [
                    opsum.tile([P, VE], f32, tag=f"o{qs}", name=f"o_ps{qs}")
                    for qs in range(4)
                ]
                for _ in range(NQB)
            ]
            p_tiles = {}

            def epilogue(qb):
                # For non-final q-blocks, evacuate PSUM first (ACT and DVE in
                # parallel) so the o_ps banks free quickly — the next
                # q-block's first mm2 reuses them — then normalize from the
                # SBUF copies. The LAST q-block normalizes straight out of
                # PSUM: nothing waits on those banks, and skipping the copy
                # shortens the kernel's tail.
                o_ps = o_ps_all[qb]
                if qb < NQB - 1:
                    src = [
                        outp.tile([P, VE], f32, tag=f"of{qs}", name=f"of_{qs}")
                        for qs in range(4)
                    ]
                    for qs in range(4):
                        if qs % 2 == 0:
                            nc.scalar.copy(src[qs], o_ps[qs])
                        else:
                            nc.vector.tensor_copy(src[qs], o_ps[qs])
                else:
                    src = o_ps
                for qs in range(4):
                    recip = smallp.tile([P, 1], f32, tag="recip")
                    nc.vector.reciprocal(recip, src[qs][:, D:D + 1])
                    o_sb = outp.tile([P, D], bf16, tag="osb")
                    if qs % 2 == 0:
                        nc.vector.tensor_scalar_mul(o_sb, src[qs][:, 0:D], recip)
                    else:
                        # ACT does the other half so the epilogue runs on two
                        # engines in parallel.
                        nc.scalar.mul(o_sb, src[qs][:, 0:D], recip)
                    row0 = qb * QB + qs * P
                    # Alternate the issuing engine so the final output DMAs
                    # overlap instead of serializing on one queue.
                    eng = nc.sync if qs % 2 == 0 else nc.scalar
                    eng.dma_start(out=out_d[row0:row0 + P, :], in_=o_sb)

            def mm1_stage(g):
                qb, t = divmod(g, KT_TILES)
                s_ps = spsum.tile([P, QB], f32, tag="s")
                nc.tensor.matmul(
                    s_ps,
                    lhsT=kt_sb[:, t, 0:P],
                    rhs=qt_sb[:, qb, 0, :],
                    start=True,
                    stop=False,
                )
                nc.tensor.matmul(
                    s_ps,
                    lhsT=kt_sb[:, t, P:2 * P],
                    rhs=qt_sb[:, qb, 1, :],
                    start=False,
                    stop=True,
                )
                p_sb = pp.tile([P, QB], bf16, tag="p")
                nc.scalar.activation(p_sb, s_ps, Exp)
                nc.vector.tensor_tensor(p_sb, p_sb, mt_sb[:, qb, t, :], mult)
                p_tiles[g] = p_sb

            def mm2_stage(g):
                qb, j = divmod(g, KT_TILES)
                p_sb = p_tiles.pop(g)
                for qs in range(4):
                    nc.tensor.matmul(
                        o_ps_all[qb][qs],
                        lhsT=p_sb[:, qs * P:(qs + 1) * P],
                        rhs=vt_sb[:, j, :],
                        start=(j == 0),
                        stop=(j == KT_TILES - 1),
                    )
                if j == KT_TILES - 1:
                    epilogue(qb)

            # mm2 before mm1 within a step, so boundary epilogue work lands
            # ahead of that step's exp/mask in the ACT/DVE queues.
            for g in range(NSTAGE):
                if g >= SKEW:
                    mm2_stage(g - SKEW)
                mm1_stage(g)
            for g in range(NSTAGE - SKEW, NSTAGE):
                mm2_stage(g)

    nc.compile()
    return nc


def _get_nc():
    if "nc" not in _STATE:
        _STATE["nc"] = _build()
    return _STATE["nc"]


def _prep_inputs(K, V, Q, m):
    bf16 = ml_dtypes.bfloat16
    scale = 1.0 / np.sqrt(np.float32(D))

    # KT[p, t, c*128+k] = K[t*128+k, c*128+p]   (p = d % 128, c = d // 128)
    kt = np.ascontiguousarray(
        K.astype(np.float32).reshape(KT_TILES, P, 2, P).transpose(3, 0, 2, 1)
    ).astype(bf16).reshape(P, KT_TILES, 2 * P)

    # VT[p, t, n] = V_ext[t*128+p, n]
    vt = np.ones((NK, VE), dtype=np.float32)
    vt[:, :D] = V
    vt = np.ascontiguousarray(
        vt.astype(bf16).reshape(KT_TILES, P, VE).transpose(1, 0, 2)
    )

    # QT[p, c, q] = Q_scaled[q, c*128+p]  (per-core slice of q)
    qs_all = (Q.astype(np.float32) * scale).T.astype(bf16)  # [D, NQ]
    mt_all = np.ascontiguousarray(m.astype(np.uint8).T)     # [NK, NQ]

    in_maps = []
    for c in range(NCORES):
        q0 = c * QSH
        # QT[p, qh, c, q] = Q_scaled[q0 + qh*QB + q, c*128 + p]
        qt_c = np.ascontiguousarray(
            qs_all[:, q0:q0 + QSH]
            .reshape(2, P, NQB, QB).transpose(1, 2, 0, 3)
        )
        # MT[p, qh, t, q] = m[q0 + qh*QB + q, t*128 + p]
        mt_c = np.ascontiguousarray(
            mt_all[:, q0:q0 + QSH]
            .reshape(KT_TILES, P, NQB, QB).transpose(1, 2, 0, 3)
        )
        in_maps.append({"KT": kt, "VT": vt, "QT": qt_c, "MT": mt_c})
    return in_maps


def kernel(K, V, Q, m):
    global LAST_RESULTS
    from concourse.bass_utils import run_bass_kernel_spmd

    nc = _get_nc()
    in_maps = _prep_inputs(
        np.asarray(K), np.asarray(V), np.asarray(Q), np.asarray(m)
    )
    try:
        res = run_bass_kernel_spmd(
            nc, in_maps, core_ids=list(range(NCORES)), trace=TRACE
        )
    except Exception:
        # Profiling hook unavailable or a transient runtime failure — retry
        # once, untraced.
        os.environ.pop("BASS_TRACE", None)
        res = run_bass_kernel_spmd(
            nc, in_maps, core_ids=list(range(NCORES)), trace=False
        )
    LAST_RESULTS = res
    out = np.concatenate([res.results[c]["out"] for c in range(NCORES)], axis=0)
    return out.astype(np.float32)


# revision 34
# speedup vs baseline: 1.0043x; 1.0043x over previous
"""Masked attention kernel for Trainium2, row-parallel over 8 NeuronCores.

Reference computation (per problem):
    S   = (Q @ K^T) / sqrt(D)          [NQ, NK]
    S   = where(m, S, -1e30)
    P   = softmax(S, axis=-1)
    out = P @ V                        [NQ, D]

Strategy:
  * Shard queries across 8 cores (1024 rows each); K/V/mask-columns replicated
    or sliced appropriately. No collectives.
  * Scores are computed TRANSPOSED on-chip: S_T[k, q] = sum_d K[k,d] * Qs[q,d]
    with Qs = Q/sqrt(D) pre-scaled on host. This makes the second matmul
    (P^T as lhsT, V as rhs) transpose-free.
  * Softmax without max-subtraction (scores are ~N(0,1); exp is safe in f32),
    multiplying by the 0/1 mask after exp.
  * Denominator for free: V is extended with a ones column, so PSUM column 256
    accumulates sum_k P[q,k]; normalize with reciprocal + per-partition scale.
  * bf16 matmul inputs (f32 PSUM accumulation), f32 output.
  * Mask is stored q-block-major so the qb0 pass streams only its own half;
    input DMAs are issued smallest-first in consumption order so the k-loop
    starts as soon as the first k-tile lands (~2 us after the DMA engines
    open) instead of waiting for whole-tensor transfers.
"""

import os
import sys

import numpy as np

sys.path.insert(0, "/opt/trn_rl_repo")

import ml_dtypes

NQ, NK, D = 8192, 8192, 256
NCORES = 8
QSH = NQ // NCORES          # 1024 queries per core
P = 128
KT_TILES = NK // P          # 64 key tiles
QB = 512                    # q-block (matmul moving free dim)
NQB = QSH // QB             # 2 q-blocks per core
VE = D + 1                  # V extended with ones column

_STATE = {}
LAST_RESULTS = None
TRACE = bool(os.environ.get("BASS_TRACE"))


def _build():
    import concourse.tile as tile
    from concourse import bacc, mybir

    bf16 = mybir.dt.bfloat16
    f32 = mybir.dt.float32
    u8 = mybir.dt.uint8

    nc = bacc.Bacc("TRN2", debug=False, enable_asserts=False, num_devices=NCORES)

    # All big inputs are partition-major: [128, ..., free] so chunked
    # DMAs move large contiguous per-partition ranges.
    kt_d = nc.dram_tensor("KT", [P, KT_TILES, 2 * P], bf16, kind="ExternalInput").ap()
    vt_d = nc.dram_tensor("VT", [P, KT_TILES, VE], bf16, kind="ExternalInput").ap()
    qt_d = nc.dram_tensor("QT", [P, NQB, 2, QB], bf16, kind="ExternalInput").ap()
    mt_d = nc.dram_tensor("MT", [P, NQB, KT_TILES, QB], u8, kind="ExternalInput").ap()
    out_d = nc.dram_tensor("out", [QSH, D], bf16, kind="ExternalOutput").ap()

    Exp = mybir.ActivationFunctionType.Exp
    mult = mybir.AluOpType.mult

    with tile.TileContext(nc) as tc:
        with (
            tc.tile_pool(name="singles", bufs=1) as singles,
            tc.tile_pool(name="pp", bufs=8) as pp,
            tc.tile_pool(name="outp", bufs=6) as outp,
            tc.tile_pool(name="smallp", bufs=4) as smallp,
            tc.tile_pool(name="spsum", bufs=4, space="PSUM") as spsum,
            tc.tile_pool(name="opsum", bufs=1, space="PSUM") as opsum,
        ):
            # First priming chunk on the scalar HWDGE ring, issued before
            # the ACT table load so it runs in parallel with sync's qt0.
            kt_sb = singles.tile([P, KT_TILES, 2 * P], bf16)
            nc.scalar.dma_start(out=kt_sb[:, 0:2, :], in_=kt_d[:, 0:2, :])

            # Prewarm the ACT exp table so its ~2.7us load overlaps input DMA.
            warm = singles.tile([P, 1], f32)
            nc.vector.memset(warm, 0.0)
            warm2 = smallp.tile([P, 1], f32, tag="warm2")
            nc.scalar.activation(warm2, warm, Exp)

            # Warm the PE HAM clock gate while the first input chunks stream
            # in: ~3.6us of continuous dummy matmuls so the HAM SHORT window
            # flips to 2.4 GHz right as the first k-tile's data lands. The
            # dummy matmuls read a raw (untracked, uninitialized) SBUF tensor
            # so they have no input dependencies; they target the opsum 'o0'
            # tile (whose first real use, an accumulation start, happens ~2 us
            # later) so the score-psum pool is never blocked.
            wdummy = nc.alloc_sbuf_tensor("wdummy", [P, QB], bf16).ap()
            warm_ps = opsum.tile([P, VE], f32, tag="o0", name="warm_ps")
            NWARM = 16
            for i in range(NWARM):
                nc.tensor.matmul(
                    warm_ps, lhsT=wdummy[:, 0:P], rhs=wdummy[:, 0:VE],
                    start=(i == 0), stop=(i == NWARM - 1),
                )

            qt_sb = singles.tile([P, NQB, 2, QB], bf16)
            vt_sb = singles.tile([P, KT_TILES, VE], bf16)
            mt_sb = singles.tile([P, NQB, KT_TILES, QB], u8)

            # Input DMAs on two queues, each in strict consumption order.
            # While the DMA path ramps (first ~6us it only sustains
            # ~150-250 GB/s), the sync queue carries just qt+kt — the stream
            # that gates mm1 directly (~75 GB/s steady). The mask and V
            # streams feed the lagging exp/mm2 stages; they ride gpsimd's
            # queue, gated open only after qt0 lands so packet-grain
            # round-robin can't slow the priming. Steady-state each queue
            # gets ~179 GB/s — both streams fit. qt1/mt1 are only needed by
            # the qb1 pass ~55us in, so they sit at the back of their queues.
            def kt_dma(a, b):
                nc.sync.dma_start(out=kt_sb[:, a:b, :], in_=kt_d[:, a:b, :])

            def vt_dma(a, b):
                nc.gpsimd.dma_start(
                    out=vt_sb[:, a:b, :], in_=vt_d[:, a:b, :]
                )

            def mt_dma(qh, a, b):
                nc.gpsimd.dma_start(
                    out=mt_sb[:, qh, a:b, :], in_=mt_d[:, qh, a:b, :]
                )

            nc.sync.dma_start(out=qt_sb[:, 0, :, :], in_=qt_d[:, 0, :, :])
            kt_dma(2, 6)
            kt_dma(6, 14)
            kt_dma(14, 22)
            # qb1 half of Q — not needed until the second pass.
            nc.sync.dma_start(out=qt_sb[:, 1, :, :], in_=qt_d[:, 1, :, :])
            for a, b in [(22, 30), (30, 38), (38, 46), (46, 54), (54, 64)]:
                kt_dma(a, b)

            # One real gate: a tiny gpsimd copy that waits for qt0 (reads
            # qt_sb) and scribbles into the first mask DMA's corner (the DMA
            # overwrites it). Runtime: the gpsimd engine stalls here, so its
            # whole DMA FIFO opens only after the priming lands. The
            # tile_wait_until hints keep the scheduler from hoisting later
            # gpsimd DMAs ahead of the gate in the emitted program.
            nc.gpsimd.tensor_copy(mt_sb[0:1, 0, 0, 0:2], qt_sb[0:1, 0, 0, 0:2])
            wi = 0

            def gp_seq():
                nonlocal_ms = 0.01 + 0.0005 * gp_seq.i
                gp_seq.i += 1
                return tc.tile_wait_until(nonlocal_ms)
            gp_seq.i = 0

            with gp_seq():
                mt_dma(0, 0, 2)
            with gp_seq():
                vt_dma(0, 2)
            with gp_seq():
                mt_dma(0, 2, 6)
            with gp_seq():
                vt_dma(2, 6)
            groups = [(6, 14), (14, 22), (22, 30), (30, 38),
                      (38, 46), (46, 54), (54, 64)]
            for gi, (a, b) in enumerate(groups):
                with gp_seq():
                    mt_dma(0, a, b)
                with gp_seq():
                    vt_dma(a, b)
                if gi == 4:
                    with gp_seq():
                        mt_dma(1, 0, 16)
                elif gi == 5:
                    with gp_seq():
                        mt_dma(1, 16, 32)
            with gp_seq():
                mt_dma(1, 32, 48)
            with gp_seq():
                mt_dma(1, 48, 64)

            # Software pipelining across BOTH q-block passes: mm1/exp/mask
            # run SKEW stages ahead of mm2 in one global stream, so the PE
            # never waits on the ~1.9us mm1 -> exp -> mask chain — not even
            # at the qb0/qb1 boundary (qb0's mm2 tail hides under qb1's
            # first mm1 tiles, and qb0's epilogue runs on ACT/DVE while the
            # PE streams on).
            SKEW = 4
            NSTAGE = NQB * KT_TILES
            o_ps_all = [
                [
                    opsum.tile([P, VE], f32, tag=f"o{qs}", name=f"o_ps{qs}")
                    for qs in range(4)
                ]
                for _ in range(NQB)
            ]
            p_tiles = {}

            def epilogue(qb):
                # For non-final q-blocks, evacuate PSUM first (ACT and DVE in
                # parallel) so the o_ps banks free quickly — the next
                # q-block's first mm2 reuses them — then normalize from the
                # SBUF copies. The LAST q-block normalizes straight out of
                # PSUM: nothing waits on those banks, and skipping the copy
                # shortens the kernel's tail.
                o_ps = o_ps_all[qb]
                if qb < NQB - 1:
                    src = [
                        outp.tile([P, VE], f32, tag=f"of{qs}", name=f"of_{qs}")
                        for qs in range(4)
                    ]
                    for qs in range(4):
                        if qs % 2 == 0:
                            nc.scalar.copy(src[qs], o_ps[qs])
                        else:
                            nc.vector.tensor_copy(src[qs], o_ps[qs])
                else:
                    src = o_ps
                for qs in range(4):
                    recip = smallp.tile([P, 1], f32, tag="recip")
                    nc.vector.reciprocal(recip, src[qs][:, D:D + 1])
                    o_sb = outp.tile([P, D], bf16, tag="osb")
                    if qs % 2 == 0:
                        nc.vector.tensor_scalar_mul(o_sb, src[qs][:, 0:D], recip)
                    else:
                        # ACT does the other half so the epilogue runs on two
                        # engines in parallel.
                        nc.scalar.mul(o_sb, src[qs][:, 0:D], recip)
                    row0 = qb * QB + qs * P
                    # Alternate the issuing engine so the final output DMAs
                    # overlap instead of serializing on one queue.
                    eng = nc.sync if qs % 2 == 0 else nc.scalar
                    eng.dma_start(out=out_d[row0:row0 + P, :], in_=o_sb)

            s_tiles = {}

            def mm1_half(g, half):
                qb, t = divmod(g, KT_TILES)
                if half == 0:
                    s_ps = spsum.tile([P, QB], f32, tag="s")
                    s_tiles[g] = s_ps
                    nc.tensor.matmul(
                        s_ps,
                        lhsT=kt_sb[:, t, 0:P],
                        rhs=qt_sb[:, qb, 0, :],
                        start=True,
                        stop=False,
                    )
                else:
                    s_ps = s_tiles.pop(g)
                    nc.tensor.matmul(
                        s_ps,
                        lhsT=kt_sb[:, t, P:2 * P],
                        rhs=qt_sb[:, qb, 1, :],
                        start=False,
                        stop=True,
                    )
                    p_sb = pp.tile([P, QB], bf16, tag="p")
                    nc.scalar.activation(p_sb, s_ps, Exp)
                    nc.vector.tensor_tensor(
                        p_sb, p_sb, mt_sb[:, qb, t, :], mult
                    )
                    p_tiles[g] = p_sb

            def mm2_half(g, half):
                qb, j = divmod(g, KT_TILES)
                p_sb = p_tiles[g] if half == 0 else p_tiles.pop(g)
                for qs in (2 * half, 2 * half + 1):
                    nc.tensor.matmul(
                        o_ps_all[qb][qs],
                        lhsT=p_sb[:, qs * P:(qs + 1) * P],
                        rhs=vt_sb[:, j, :],
                        start=(j == 0),
                        stop=(j == KT_TILES - 1),
                    )
                if half == 1 and j == KT_TILES - 1:
                    epilogue(qb)

            # Within a step, mm1's two 512-cycle matmuls are interleaved
            # between the mm2 LDW+MM pairs: each big matmul is a recovery
            # window in which the next weight loads hide with slack,
            # instead of four back-to-back zero-slack LDW+MM(257) pairs.
            # mm2 before mm1 so boundary epilogue work lands ahead of that
            # step's exp/mask in the ACT/DVE queues.
            for g in range(NSTAGE):
                if g >= SKEW:
                    mm2_half(g - SKEW, 0)
                mm1_half(g, 0)
                if g >= SKEW:
                    mm2_half(g - SKEW, 1)
                mm1_half(g, 1)
            for g in range(NSTAGE - SKEW, NSTAGE):
                mm2_half(g, 0)
                mm2_half(g, 1)

    nc.compile()
    return nc


def _get_nc():
    if "nc" not in _STATE:
        _STATE["nc"] = _build()
    return _STATE["nc"]


def _prep_inputs(K, V, Q, m):
    bf16 = ml_dtypes.bfloat16
    scale = 1.0 / np.sqrt(np.float32(D))

    # KT[p, t, c*128+k] = K[t*128+k, c*128+p]   (p = d % 128, c = d // 128)
    kt = np.ascontiguousarray(
        K.astype(np.float32).reshape(KT_TILES, P, 2, P).transpose(3, 0, 2, 1)
    ).astype(bf16).reshape(P, KT_TILES, 2 * P)

    # VT[p, t, n] = V_ext[t*128+p, n]
    vt = np.ones((NK, VE), dtype=np.float32)
    vt[:, :D] = V
    vt = np.ascontiguousarray(
        vt.astype(bf16).reshape(KT_TILES, P, VE).transpose(1, 0, 2)
    )

    # QT[p, c, q] = Q_scaled[q, c*128+p]  (per-core slice of q)
    qs_all = (Q.astype(np.float32) * scale).T.astype(bf16)  # [D, NQ]
    mt_all = np.ascontiguousarray(m.astype(np.uint8).T)     # [NK, NQ]

    in_maps = []
    for c in range(NCORES):
        q0 = c * QSH
        # QT[p, qh, c, q] = Q_scaled[q0 + qh*QB + q, c*128 + p]
        qt_c = np.ascontiguousarray(
            qs_all[:, q0:q0 + QSH]
            .reshape(2, P, NQB, QB).transpose(1, 2, 0, 3)
        )
        # MT[p, qh, t, q] = m[q0 + qh*QB + q, t*128 + p]
        mt_c = np.ascontiguousarray(
            mt_all[:, q0:q0 + QSH]
            .reshape(KT_TILES, P, NQB, QB).transpose(1, 2, 0, 3)
        )
        in_maps.append({"KT": kt, "VT": vt, "QT": qt_c, "MT": mt_c})
    return in_maps


def kernel(K, V, Q, m):
    global LAST_RESULTS
    from concourse.bass_utils import run_bass_kernel_spmd

    nc = _get_nc()
    in_maps = _prep_inputs(
        np.asarray(K), np.asarray(V), np.asarray(Q), np.asarray(m)
    )
    try:
        res = run_bass_kernel_spmd(
            nc, in_maps, core_ids=list(range(NCORES)), trace=TRACE
        )
    except Exception:
        # Profiling hook unavailable or a transient runtime failure — retry
        # once, untraced.
        os.environ.pop("BASS_TRACE", None)
        res = run_bass_kernel_spmd(
            nc, in_maps, core_ids=list(range(NCORES)), trace=False
        )
    LAST_RESULTS = res
    out = np.concatenate([res.results[c]["out"] for c in range(NCORES)], axis=0)
    return out.astype(np.float32)


# revision 35
# speedup vs baseline: 1.0142x; 1.0099x over previous
"""Masked attention kernel for Trainium2, row-parallel over 8 NeuronCores.

Reference computation (per problem):
    S   = (Q @ K^T) / sqrt(D)          [NQ, NK]
    S   = where(m, S, -1e30)
    P   = softmax(S, axis=-1)
    out = P @ V                        [NQ, D]

Strategy:
  * Shard queries across 8 cores (1024 rows each); K/V/mask-columns replicated
    or sliced appropriately. No collectives.
  * Scores are computed TRANSPOSED on-chip: S_T[k, q] = sum_d K[k,d] * Qs[q,d]
    with Qs = Q/sqrt(D) pre-scaled on host. This makes the second matmul
    (P^T as lhsT, V as rhs) transpose-free.
  * Softmax without max-subtraction (scores are ~N(0,1); exp is safe in f32),
    multiplying by the 0/1 mask after exp.
  * Denominator for free: V is extended with a ones column, so PSUM column 256
    accumulates sum_k P[q,k]; normalize with reciprocal + per-partition scale.
  * bf16 matmul inputs (f32 PSUM accumulation), bf16 output DMA (cast to
    f32 on host; adds ~2.3e-3 error in quadrature, halves output traffic).
  * Mask is stored q-block-major so the qb0 pass streams only its own half;
    input DMAs are issued smallest-first in consumption order so the k-loop
    starts as soon as the first k-tile lands (~2 us after the DMA engines
    open) instead of waiting for whole-tensor transfers.
"""

import os
import sys

import numpy as np

sys.path.insert(0, "/opt/trn_rl_repo")

import ml_dtypes

NQ, NK, D = 8192, 8192, 256
NCORES = 8
QSH = NQ // NCORES          # 1024 queries per core
P = 128
KT_TILES = NK // P          # 64 key tiles
QB = 512                    # q-block (matmul moving free dim)
NQB = QSH // QB             # 2 q-blocks per core
VE = D + 1                  # V extended with ones column

_STATE = {}
LAST_RESULTS = None
TRACE = bool(os.environ.get("BASS_TRACE"))


def _build():
    import concourse.tile as tile
    from concourse import bacc, mybir

    bf16 = mybir.dt.bfloat16
    f32 = mybir.dt.float32
    u8 = mybir.dt.uint8

    nc = bacc.Bacc("TRN2", debug=False, enable_asserts=False, num_devices=NCORES)

    # All big inputs are partition-major: [128, ..., free] so chunked
    # DMAs move large contiguous per-partition ranges.
    kt_d = nc.dram_tensor("KT", [P, KT_TILES, 2 * P], bf16, kind="ExternalInput").ap()
    vt_d = nc.dram_tensor("VT", [P, KT_TILES, VE], bf16, kind="ExternalInput").ap()
    qt_d = nc.dram_tensor("QT", [P, NQB, 2, QB], bf16, kind="ExternalInput").ap()
    mt_d = nc.dram_tensor("MT", [P, NQB, KT_TILES, QB], u8, kind="ExternalInput").ap()
    out_d = nc.dram_tensor("out", [QSH, D], bf16, kind="ExternalOutput").ap()

    Exp = mybir.ActivationFunctionType.Exp
    mult = mybir.AluOpType.mult

    with tile.TileContext(nc) as tc:
        with (
            tc.tile_pool(name="singles", bufs=1) as singles,
            tc.tile_pool(name="pp", bufs=8) as pp,
            tc.tile_pool(name="outp", bufs=6) as outp,
            tc.tile_pool(name="smallp", bufs=4) as smallp,
            tc.tile_pool(name="spsum", bufs=4, space="PSUM") as spsum,
            tc.tile_pool(name="opsum", bufs=1, space="PSUM") as opsum,
        ):
            # First priming chunk on the scalar HWDGE ring, issued before
            # the ACT table load so it runs in parallel with sync's qt0.
            kt_sb = singles.tile([P, KT_TILES, 2 * P], bf16)
            nc.scalar.dma_start(out=kt_sb[:, 0:2, :], in_=kt_d[:, 0:2, :])

            # Prewarm the ACT exp table so its ~2.7us load overlaps input DMA.
            warm = singles.tile([P, 1], f32)
            nc.vector.memset(warm, 0.0)
            warm2 = smallp.tile([P, 1], f32, tag="warm2")
            nc.scalar.activation(warm2, warm, Exp)

            # Warm the PE HAM clock gate while the first input chunks stream
            # in: ~3.6us of continuous dummy matmuls so the HAM SHORT window
            # flips to 2.4 GHz right as the first k-tile's data lands. The
            # dummy matmuls read a raw (untracked, uninitialized) SBUF tensor
            # so they have no input dependencies; they target the opsum 'o0'
            # tile (whose first real use, an accumulation start, happens ~2 us
            # later) so the score-psum pool is never blocked.
            wdummy = nc.alloc_sbuf_tensor("wdummy", [P, QB], bf16).ap()
            warm_ps = opsum.tile([P, VE], f32, tag="o0", name="warm_ps")
            NWARM = 16
            for i in range(NWARM):
                nc.tensor.matmul(
                    warm_ps, lhsT=wdummy[:, 0:P], rhs=wdummy[:, 0:VE],
                    start=(i == 0), stop=(i == NWARM - 1),
                )

            qt_sb = singles.tile([P, NQB, 2, QB], bf16)
            vt_sb = singles.tile([P, KT_TILES, VE], bf16)
            mt_sb = singles.tile([P, NQB, KT_TILES, QB], u8)

            # Input DMAs on two queues, each in strict consumption order.
            # While the DMA path ramps (first ~6us it only sustains
            # ~150-250 GB/s), the sync queue carries just qt+kt — the stream
            # that gates mm1 directly (~75 GB/s steady). The mask and V
            # streams feed the lagging exp/mm2 stages; they ride gpsimd's
            # queue, gated open only after qt0 lands so packet-grain
            # round-robin can't slow the priming. Steady-state each queue
            # gets ~179 GB/s — both streams fit. qt1/mt1 are only needed by
            # the qb1 pass ~55us in, so they sit at the back of their queues.
            def kt_dma(a, b):
                nc.sync.dma_start(out=kt_sb[:, a:b, :], in_=kt_d[:, a:b, :])

            def vt_dma(a, b):
                nc.gpsimd.dma_start(
                    out=vt_sb[:, a:b, :], in_=vt_d[:, a:b, :]
                )

            def mt_dma(qh, a, b):
                nc.gpsimd.dma_start(
                    out=mt_sb[:, qh, a:b, :], in_=mt_d[:, qh, a:b, :]
                )

            nc.sync.dma_start(out=qt_sb[:, 0, :, :], in_=qt_d[:, 0, :, :])
            kt_dma(2, 6)
            kt_dma(6, 14)
            kt_dma(14, 22)
            # qb1 half of Q — not needed until the second pass.
            nc.sync.dma_start(out=qt_sb[:, 1, :, :], in_=qt_d[:, 1, :, :])
            for a, b in [(22, 30), (30, 38), (38, 46), (46, 54), (54, 64)]:
                kt_dma(a, b)

            # One real gate: a tiny gpsimd copy that waits for qt0 (reads
            # qt_sb) and scribbles into the first mask DMA's corner (the DMA
            # overwrites it). Runtime: the gpsimd engine stalls here, so its
            # whole DMA FIFO opens only after the priming lands. The
            # tile_wait_until hints keep the scheduler from hoisting later
            # gpsimd DMAs ahead of the gate in the emitted program.
            nc.gpsimd.tensor_copy(mt_sb[0:1, 0, 0, 0:2], qt_sb[0:1, 0, 0, 0:2])
            wi = 0

            def gp_seq():
                nonlocal_ms = 0.01 + 0.0005 * gp_seq.i
                gp_seq.i += 1
                return tc.tile_wait_until(nonlocal_ms)
            gp_seq.i = 0

            with gp_seq():
                mt_dma(0, 0, 2)
            with gp_seq():
                vt_dma(0, 2)
            with gp_seq():
                mt_dma(0, 2, 6)
            with gp_seq():
                vt_dma(2, 6)
            groups = [(6, 14), (14, 22), (22, 30), (30, 38),
                      (38, 46), (46, 54), (54, 64)]
            for gi, (a, b) in enumerate(groups):
                with gp_seq():
                    mt_dma(0, a, b)
                with gp_seq():
                    vt_dma(a, b)
                if gi == 4:
                    with gp_seq():
                        mt_dma(1, 0, 16)
                elif gi == 5:
                    with gp_seq():
                        mt_dma(1, 16, 32)
            with gp_seq():
                mt_dma(1, 32, 48)
            with gp_seq():
                mt_dma(1, 48, 64)

            # Software pipelining across BOTH q-block passes: mm1/exp/mask
            # run SKEW stages ahead of mm2 in one global stream, so the PE
            # never waits on the ~1.9us mm1 -> exp -> mask chain — not even
            # at the qb0/qb1 boundary (qb0's mm2 tail hides under qb1's
            # first mm1 tiles, and qb0's epilogue runs on ACT/DVE while the
            # PE streams on).
            SKEW = 4
            NSTAGE = NQB * KT_TILES
            o_ps_all = [
                [
                    opsum.tile([P, VE], f32, tag=f"o{qs}", name=f"o_ps{qs}")
                    for qs in range(4)
                ]
                for _ in range(NQB)
            ]
            p_tiles = {}

            def epilogue(qb):
                # For non-final q-blocks, evacuate PSUM first (ACT and DVE in
                # parallel) so the o_ps banks free quickly — the next
                # q-block's first mm2 reuses them — then normalize from the
                # SBUF copies. The LAST q-block normalizes straight out of
                # PSUM: nothing waits on those banks, and skipping the copy
                # shortens the kernel's tail.
                o_ps = o_ps_all[qb]
                if qb < NQB - 1:
                    src = [
                        outp.tile([P, VE], f32, tag=f"of{qs}", name=f"of_{qs}")
                        for qs in range(4)
                    ]
                    for qs in range(4):
                        if qs % 2 == 0:
                            nc.scalar.copy(src[qs], o_ps[qs])
                        else:
                            nc.vector.tensor_copy(src[qs], o_ps[qs])
                else:
                    src = o_ps
                for qs in range(4):
                    recip = smallp.tile([P, 1], f32, tag="recip")
                    nc.vector.reciprocal(recip, src[qs][:, D:D + 1])
                    o_sb = outp.tile([P, D], bf16, tag="osb")
                    if qs % 2 == 0:
                        nc.vector.tensor_scalar_mul(o_sb, src[qs][:, 0:D], recip)
                    else:
                        # ACT does the other half so the epilogue runs on two
                        # engines in parallel.
                        nc.scalar.mul(o_sb, src[qs][:, 0:D], recip)
                    row0 = qb * QB + qs * P
                    # Alternate the issuing engine so the final output DMAs
                    # overlap instead of serializing on one queue.
                    eng = nc.sync if qs % 2 == 0 else nc.scalar
                    eng.dma_start(out=out_d[row0:row0 + P, :], in_=o_sb)

            s_tiles = {}

            def mm1_half(g, half):
                qb, t = divmod(g, KT_TILES)
                if half == 0:
                    s_ps = spsum.tile([P, QB], f32, tag="s")
                    s_tiles[g] = s_ps
                    nc.tensor.matmul(
                        s_ps,
                        lhsT=kt_sb[:, t, 0:P],
                        rhs=qt_sb[:, qb, 0, :],
                        start=True,
                        stop=False,
                    )
                else:
                    s_ps = s_tiles.pop(g)
                    nc.tensor.matmul(
                        s_ps,
                        lhsT=kt_sb[:, t, P:2 * P],
                        rhs=qt_sb[:, qb, 1, :],
                        start=False,
                        stop=True,
                    )
                    p_sb = pp.tile([P, QB], bf16, tag="p")
                    nc.scalar.activation(p_sb, s_ps, Exp)
                    nc.vector.tensor_tensor(
                        p_sb, p_sb, mt_sb[:, qb, t, :], mult
                    )
                    p_tiles[g] = p_sb

            def mm2_half(g, half):
                qb, j = divmod(g, KT_TILES)
                p_sb = p_tiles[g] if half == 0 else p_tiles.pop(g)
                for qs in (2 * half, 2 * half + 1):
                    nc.tensor.matmul(
                        o_ps_all[qb][qs],
                        lhsT=p_sb[:, qs * P:(qs + 1) * P],
                        rhs=vt_sb[:, j, :],
                        start=(j == 0),
                        stop=(j == KT_TILES - 1),
                    )
                if half == 1 and j == KT_TILES - 1:
                    epilogue(qb)

            # Within a step, mm1's two 512-cycle matmuls are interleaved
            # between the mm2 LDW+MM pairs: each big matmul is a recovery
            # window in which the next weight loads hide with slack,
            # instead of four back-to-back zero-slack LDW+MM(257) pairs.
            # mm2 before mm1 so boundary epilogue work lands ahead of that
            # step's exp/mask in the ACT/DVE queues.
            for g in range(NSTAGE):
                if g >= SKEW:
                    mm2_half(g - SKEW, 0)
                mm1_half(g, 0)
                if g >= SKEW:
                    mm2_half(g - SKEW, 1)
                mm1_half(g, 1)
            for g in range(NSTAGE - SKEW, NSTAGE):
                mm2_half(g, 0)
                mm2_half(g, 1)

    nc.compile()
    return nc


def _get_nc():
    if "nc" not in _STATE:
        _STATE["nc"] = _build()
    return _STATE["nc"]


def _prep_inputs(K, V, Q, m):
    bf16 = ml_dtypes.bfloat16
    scale = 1.0 / np.sqrt(np.float32(D))

    # KT[p, t, c*128+k] = K[t*128+k, c*128+p]   (p = d % 128, c = d // 128)
    kt = np.ascontiguousarray(
        K.astype(np.float32).reshape(KT_TILES, P, 2, P).transpose(3, 0, 2, 1)
    ).astype(bf16).reshape(P, KT_TILES, 2 * P)

    # VT[p, t, n] = V_ext[t*128+p, n]
    vt = np.ones((NK, VE), dtype=np.float32)
    vt[:, :D] = V
    vt = np.ascontiguousarray(
        vt.astype(bf16).reshape(KT_TILES, P, VE).transpose(1, 0, 2)
    )

    # QT[p, c, q] = Q_scaled[q, c*128+p]  (per-core slice of q)
    qs_all = (Q.astype(np.float32) * scale).T.astype(bf16)  # [D, NQ]
    mt_all = np.ascontiguousarray(m.astype(np.uint8).T)     # [NK, NQ]

    in_maps = []
    for c in range(NCORES):
        q0 = c * QSH
        # QT[p, qh, c, q] = Q_scaled[q0 + qh*QB + q, c*128 + p]
        qt_c = np.ascontiguousarray(
            qs_all[:, q0:q0 + QSH]
            .reshape(2, P, NQB, QB).transpose(1, 2, 0, 3)
        )
        # MT[p, qh, t, q] = m[q0 + qh*QB + q, t*128 + p]
        mt_c = np.ascontiguousarray(
            mt_all[:, q0:q0 + QSH]
            .reshape(KT_TILES, P, NQB, QB).transpose(1, 2, 0, 3)
        )
        in_maps.append({"KT": kt, "VT": vt, "QT": qt_c, "MT": mt_c})
    return in_maps


def kernel(K, V, Q, m):
    global LAST_RESULTS
    from concourse.bass_utils import run_bass_kernel_spmd

    nc = _get_nc()
    in_maps = _prep_inputs(
        np.asarray(K), np.asarray(V), np.asarray(Q), np.asarray(m)
    )
    try:
        res = run_bass_kernel_spmd(
            nc, in_maps, core_ids=list(range(NCORES)), trace=TRACE
        )
    except Exception:
        # Profiling hook unavailable or a transient runtime failure — retry
        # once, untraced.
        os.environ.pop("BASS_TRACE", None)
        res = run_bass_kernel_spmd(
            nc, in_maps, core_ids=list(range(NCORES)), trace=False
        )
    LAST_RESULTS = res
    out = np.concatenate([res.results[c]["out"] for c in range(NCORES)], axis=0)
    return out.astype(np.float32)


# revision 37
# speedup vs baseline: 1.0178x; 1.0035x over previous
"""Masked attention kernel for Trainium2, row-parallel over 8 NeuronCores.

Reference computation (per problem):
    S   = (Q @ K^T) / sqrt(D)          [NQ, NK]
    S   = where(m, S, -1e30)
    P   = softmax(S, axis=-1)
    out = P @ V                        [NQ, D]

Strategy:
  * Shard queries across 8 cores (1024 rows each); K/V/mask-columns replicated
    or sliced appropriately. No collectives.
  * Scores are computed TRANSPOSED on-chip: S_T[k, q] = sum_d K[k,d] * Qs[q,d]
    with Qs = Q/sqrt(D) pre-scaled on host. This makes the second matmul
    (P^T as lhsT, V as rhs) transpose-free.
  * Softmax without max-subtraction (scores are ~N(0,1); exp is safe in f32),
    multiplying by the 0/1 mask after exp.
  * Denominator for free: V is extended with a ones column, so PSUM column 256
    accumulates sum_k P[q,k]; normalize with reciprocal + per-partition scale.
  * bf16 matmul inputs (f32 PSUM accumulation), bf16 output DMA (cast to
    f32 on host; adds ~2.3e-3 error in quadrature, halves output traffic).
  * Mask is stored q-block-major so the qb0 pass streams only its own half;
    input DMAs are issued smallest-first in consumption order so the k-loop
    starts as soon as the first k-tile lands (~2 us after the DMA engines
    open) instead of waiting for whole-tensor transfers.
"""

import os
import sys

import numpy as np

sys.path.insert(0, "/opt/trn_rl_repo")

import ml_dtypes

NQ, NK, D = 8192, 8192, 256
NCORES = 8
QSH = NQ // NCORES          # 1024 queries per core
P = 128
KT_TILES = NK // P          # 64 key tiles
QB = 512                    # q-block (matmul moving free dim)
NQB = QSH // QB             # 2 q-blocks per core
VE = D + 1                  # V extended with ones column

_STATE = {}
LAST_RESULTS = None
TRACE = bool(os.environ.get("BASS_TRACE"))


def _build():
    import concourse.tile as tile
    from concourse import bacc, mybir

    bf16 = mybir.dt.bfloat16
    f32 = mybir.dt.float32
    u8 = mybir.dt.uint8

    nc = bacc.Bacc("TRN2", debug=False, enable_asserts=False, num_devices=NCORES)

    # All big inputs are partition-major: [128, ..., free] so chunked
    # DMAs move large contiguous per-partition ranges.
    kt_d = nc.dram_tensor("KT", [P, KT_TILES, 2 * P], bf16, kind="ExternalInput").ap()
    vt_d = nc.dram_tensor("VT", [P, KT_TILES, VE], bf16, kind="ExternalInput").ap()
    qt_d = nc.dram_tensor("QT", [P, NQB, 2, QB], bf16, kind="ExternalInput").ap()
    mt_d = nc.dram_tensor("MT", [P, NQB, KT_TILES, QB], u8, kind="ExternalInput").ap()
    out_d = nc.dram_tensor("out", [QSH, D], bf16, kind="ExternalOutput").ap()

    Exp = mybir.ActivationFunctionType.Exp
    mult = mybir.AluOpType.mult

    with tile.TileContext(nc) as tc:
        with (
            tc.tile_pool(name="singles", bufs=1) as singles,
            tc.tile_pool(name="pp", bufs=8) as pp,
            tc.tile_pool(name="outp", bufs=6) as outp,
            tc.tile_pool(name="smallp", bufs=4) as smallp,
            tc.tile_pool(name="spsum", bufs=4, space="PSUM") as spsum,
            tc.tile_pool(name="opsum", bufs=1, space="PSUM") as opsum,
        ):
            # First priming chunk on the scalar HWDGE ring, issued before
            # the ACT table load so it runs in parallel with sync's qt0.
            kt_sb = singles.tile([P, KT_TILES, 2 * P], bf16)
            nc.scalar.dma_start(out=kt_sb[:, 0:2, :], in_=kt_d[:, 0:2, :])

            # Prewarm the ACT exp table so its ~2.7us load overlaps input DMA.
            warm = singles.tile([P, 1], f32)
            nc.vector.memset(warm, 0.0)
            warm2 = smallp.tile([P, 1], f32, tag="warm2")
            nc.scalar.activation(warm2, warm, Exp)

            # Warm the PE HAM clock gate while the first input chunks stream
            # in: ~3.6us of continuous dummy matmuls so the HAM SHORT window
            # flips to 2.4 GHz right as the first k-tile's data lands. The
            # dummy matmuls read a raw (untracked, uninitialized) SBUF tensor
            # so they have no input dependencies; they target the opsum 'o0'
            # tile (whose first real use, an accumulation start, happens ~2 us
            # later) so the score-psum pool is never blocked.
            wdummy = nc.alloc_sbuf_tensor("wdummy", [P, QB], bf16).ap()
            warm_ps = opsum.tile([P, VE], f32, tag="o0", name="warm_ps")
            NWARM = 16
            for i in range(NWARM):
                nc.tensor.matmul(
                    warm_ps, lhsT=wdummy[:, 0:P], rhs=wdummy[:, 0:VE],
                    start=(i == 0), stop=(i == NWARM - 1),
                )

            qt_sb = singles.tile([P, NQB, 2, QB], bf16)
            vt_sb = singles.tile([P, KT_TILES, VE], bf16)
            mt_sb = singles.tile([P, NQB, KT_TILES, QB], u8)

            # Input DMAs on two queues, each in strict consumption order.
            # While the DMA path ramps (first ~6us it only sustains
            # ~150-250 GB/s), the sync queue carries just qt+kt — the stream
            # that gates mm1 directly (~75 GB/s steady). The mask and V
            # streams feed the lagging exp/mm2 stages; they ride gpsimd's
            # queue, gated open only after qt0 lands so packet-grain
            # round-robin can't slow the priming. Steady-state each queue
            # gets ~179 GB/s — both streams fit. qt1/mt1 are only needed by
            # the qb1 pass ~55us in, so they sit at the back of their queues.
            def kt_dma(a, b):
                nc.sync.dma_start(out=kt_sb[:, a:b, :], in_=kt_d[:, a:b, :])

            def vt_dma(a, b):
                nc.gpsimd.dma_start(
                    out=vt_sb[:, a:b, :], in_=vt_d[:, a:b, :]
                )

            def mt_dma(qh, a, b):
                nc.gpsimd.dma_start(
                    out=mt_sb[:, qh, a:b, :], in_=mt_d[:, qh, a:b, :]
                )

            nc.sync.dma_start(out=qt_sb[:, 0, :, :], in_=qt_d[:, 0, :, :])
            kt_dma(2, 6)
            kt_dma(6, 14)
            kt_dma(14, 22)
            # qb1 half of Q — not needed until the second pass.
            nc.sync.dma_start(out=qt_sb[:, 1, :, :], in_=qt_d[:, 1, :, :])
            for a, b in [(22, 30), (30, 38), (38, 46), (46, 54), (54, 64)]:
                kt_dma(a, b)

            # One real gate: a tiny gpsimd copy that waits for qt0 (reads
            # qt_sb) and scribbles into the first mask DMA's corner (the DMA
            # overwrites it). Runtime: the gpsimd engine stalls here, so its
            # whole DMA FIFO opens only after the priming lands. The
            # tile_wait_until hints keep the scheduler from hoisting later
            # gpsimd DMAs ahead of the gate in the emitted program.
            nc.gpsimd.tensor_copy(mt_sb[0:1, 0, 0, 0:2], qt_sb[0:1, 0, 0, 0:2])
            wi = 0

            def gp_seq():
                nonlocal_ms = 0.01 + 0.0005 * gp_seq.i
                gp_seq.i += 1
                return tc.tile_wait_until(nonlocal_ms)
            gp_seq.i = 0

            with gp_seq():
                mt_dma(0, 0, 2)
            with gp_seq():
                vt_dma(0, 2)
            with gp_seq():
                mt_dma(0, 2, 6)
            with gp_seq():
                vt_dma(2, 6)
            groups = [(6, 14), (14, 22), (22, 30), (30, 38),
                      (38, 46), (46, 54), (54, 64)]
            for gi, (a, b) in enumerate(groups):
                with gp_seq():
                    mt_dma(0, a, b)
                with gp_seq():
                    vt_dma(a, b)
                if gi == 4:
                    with gp_seq():
                        mt_dma(1, 0, 16)
                elif gi == 5:
                    with gp_seq():
                        mt_dma(1, 16, 32)
            with gp_seq():
                mt_dma(1, 32, 48)
            with gp_seq():
                mt_dma(1, 48, 64)

            # Software pipelining across BOTH q-block passes: mm1/exp/mask
            # run SKEW stages ahead of mm2 in one global stream, so the PE
            # never waits on the ~1.9us mm1 -> exp -> mask chain — not even
            # at the qb0/qb1 boundary (qb0's mm2 tail hides under qb1's
            # first mm1 tiles, and qb0's epilogue runs on ACT/DVE while the
            # PE streams on).
            SKEW = 6
            NSTAGE = NQB * KT_TILES
            o_ps_all = [
                [
                    opsum.tile([P, VE], f32, tag=f"o{qs}", name=f"o_ps{qs}")
                    for qs in range(4)
                ]
                for _ in range(NQB)
            ]
            p_tiles = {}

            def epilogue(qb):
                # For non-final q-blocks, evacuate PSUM first (ACT and DVE in
                # parallel) so the o_ps banks free quickly — the next
                # q-block's first mm2 reuses them — then normalize from the
                # SBUF copies. The LAST q-block normalizes straight out of
                # PSUM: nothing waits on those banks, and skipping the copy
                # shortens the kernel's tail.
                o_ps = o_ps_all[qb]
                if qb < NQB - 1:
                    src = [
                        outp.tile([P, VE], f32, tag=f"of{qs}", name=f"of_{qs}")
                        for qs in range(4)
                    ]
                    for qs in range(4):
                        if qs % 2 == 0:
                            nc.scalar.copy(src[qs], o_ps[qs])
                        else:
                            nc.vector.tensor_copy(src[qs], o_ps[qs])
                else:
                    src = o_ps
                for qs in range(4):
                    recip = smallp.tile([P, 1], f32, tag="recip")
                    nc.vector.reciprocal(recip, src[qs][:, D:D + 1])
                    o_sb = outp.tile([P, D], bf16, tag="osb")
                    if qs % 2 == 0:
                        nc.vector.tensor_scalar_mul(o_sb, src[qs][:, 0:D], recip)
                    else:
                        # ACT does the other half so the epilogue runs on two
                        # engines in parallel.
                        nc.scalar.mul(o_sb, src[qs][:, 0:D], recip)
                    row0 = qb * QB + qs * P
                    # Alternate the issuing engine so the final output DMAs
                    # overlap instead of serializing on one queue.
                    eng = nc.sync if qs % 2 == 0 else nc.scalar
                    eng.dma_start(out=out_d[row0:row0 + P, :], in_=o_sb)

            s_tiles = {}

            def mm1_half(g, half):
                qb, t = divmod(g, KT_TILES)
                if half == 0:
                    s_ps = spsum.tile([P, QB], f32, tag="s")
                    s_tiles[g] = s_ps
                    nc.tensor.matmul(
                        s_ps,
                        lhsT=kt_sb[:, t, 0:P],
                        rhs=qt_sb[:, qb, 0, :],
                        start=True,
                        stop=False,
                    )
                else:
                    s_ps = s_tiles.pop(g)
                    nc.tensor.matmul(
                        s_ps,
                        lhsT=kt_sb[:, t, P:2 * P],
                        rhs=qt_sb[:, qb, 1, :],
                        start=False,
                        stop=True,
                    )
                    p_sb = pp.tile([P, QB], bf16, tag="p")
                    nc.scalar.activation(p_sb, s_ps, Exp)
                    nc.vector.tensor_tensor(
                        p_sb, p_sb, mt_sb[:, qb, t, :], mult
                    )
                    p_tiles[g] = p_sb

            def mm2_half(g, half):
                qb, j = divmod(g, KT_TILES)
                p_sb = p_tiles[g] if half == 0 else p_tiles.pop(g)
                for qs in (2 * half, 2 * half + 1):
                    nc.tensor.matmul(
                        o_ps_all[qb][qs],
                        lhsT=p_sb[:, qs * P:(qs + 1) * P],
                        rhs=vt_sb[:, j, :],
                        start=(j == 0),
                        stop=(j == KT_TILES - 1),
                    )
                if half == 1 and j == KT_TILES - 1:
                    epilogue(qb)

            # Within a step, mm1's two 512-cycle matmuls are interleaved
            # between the mm2 LDW+MM pairs: each big matmul is a recovery
            # window in which the next weight loads hide with slack,
            # instead of four back-to-back zero-slack LDW+MM(257) pairs.
            # mm2 before mm1 so boundary epilogue work lands ahead of that
            # step's exp/mask in the ACT/DVE queues.
            for g in range(NSTAGE):
                if g >= SKEW:
                    mm2_half(g - SKEW, 0)
                mm1_half(g, 0)
                if g >= SKEW:
                    mm2_half(g - SKEW, 1)
                mm1_half(g, 1)
            for g in range(NSTAGE - SKEW, NSTAGE):
                mm2_half(g, 0)
                mm2_half(g, 1)

    nc.compile()
    return nc


def _get_nc():
    if "nc" not in _STATE:
        _STATE["nc"] = _build()
    return _STATE["nc"]


def _prep_inputs(K, V, Q, m):
    bf16 = ml_dtypes.bfloat16
    scale = 1.0 / np.sqrt(np.float32(D))

    # KT[p, t, c*128+k] = K[t*128+k, c*128+p]   (p = d % 128, c = d // 128)
    kt = np.ascontiguousarray(
        K.astype(np.float32).reshape(KT_TILES, P, 2, P).transpose(3, 0, 2, 1)
    ).astype(bf16).reshape(P, KT_TILES, 2 * P)

    # VT[p, t, n] = V_ext[t*128+p, n]
    vt = np.ones((NK, VE), dtype=np.float32)
    vt[:, :D] = V
    vt = np.ascontiguousarray(
        vt.astype(bf16).reshape(KT_TILES, P, VE).transpose(1, 0, 2)
    )

    # QT[p, c, q] = Q_scaled[q, c*128+p]  (per-core slice of q)
    qs_all = (Q.astype(np.float32) * scale).T.astype(bf16)  # [D, NQ]
    mt_all = np.ascontiguousarray(m.astype(np.uint8).T)     # [NK, NQ]

    in_maps = []
    for c in range(NCORES):
        q0 = c * QSH
        # QT[p, qh, c, q] = Q_scaled[q0 + qh*QB + q, c*128 + p]
        qt_c = np.ascontiguousarray(
            qs_all[:, q0:q0 + QSH]
            .reshape(2, P, NQB, QB).transpose(1, 2, 0, 3)
        )
        # MT[p, qh, t, q] = m[q0 + qh*QB + q, t*128 + p]
        mt_c = np.ascontiguousarray(
            mt_all[:, q0:q0 + QSH]
            .reshape(KT_TILES, P, NQB, QB).transpose(1, 2, 0, 3)
        )
        in_maps.append({"KT": kt, "VT": vt, "QT": qt_c, "MT": mt_c})
    return in_maps


def kernel(K, V, Q, m):
    global LAST_RESULTS
    from concourse.bass_utils import run_bass_kernel_spmd

    nc = _get_nc()
    in_maps = _prep_inputs(
        np.asarray(K), np.asarray(V), np.asarray(Q), np.asarray(m)
    )
    try:
        res = run_bass_kernel_spmd(
            nc, in_maps, core_ids=list(range(NCORES)), trace=TRACE
        )
    except Exception:
        # Profiling hook unavailable or a transient runtime failure — retry
        # once, untraced.
        os.environ.pop("BASS_TRACE", None)
        res = run_bass_kernel_spmd(
            nc, in_maps, core_ids=list(range(NCORES)), trace=False
        )
    LAST_RESULTS = res
    out = np.concatenate([res.results[c]["out"] for c in range(NCORES)], axis=0)
    return out.astype(np.float32)
